# revision 22
# baseline (speedup 1.0000x reference)
"""Differential cross-attention Bass kernel for 8 Trainium2 NeuronCores.

Sharding: heads are split across cores (2 of 16 heads per core). Each core
computes Q/K/V projections for its head slice only, both N x N differential
score maps for its (batch, head) units, softmax (no max-subtraction; scores
are O(1) so exp is safe), attn = a1 - lam*a2, GroupNorm per (b, h), and a
partial output projection against its 128-column slice of Wo. The host sums
the 8 partial outputs and adds the output bias.

Layouts on device (per core, P = SBUF partition dim):
  Q, K   dim-major   (128 dims = 2 heads x 64, B*N tokens)
  V      token-major (128 tokens/chunk, 32 chunks, 2x(64 dims + ones col))
  scores S.T         (128 k-tokens, q free)  -> exp -> P.T
  O_aug.T = V_aug.T @ P.T  (65, q)  row 64 = softmax denominator
  transpose -> (q, 65), normalize+combine per-partition, GN stats,
  transpose back -> (64, q) stacked per (b, attend) -> Wo matmul -> partial out
"""

import os
import sys
from contextlib import ExitStack

import numpy as np

for _p in ("/opt/trn_rl_repo", "/opt/pypackages"):
    if os.path.isdir(_p) and _p not in sys.path:
        sys.path.append(_p)

import concourse.bass as bass
import concourse.tile as tile
from concourse import bacc, mybir
from concourse.bass_utils import run_bass_kernel_spmd
from concourse.masks import make_identity

# ---- problem constants (hardcoded per contest contract) ----
B, N, DIM, H, HEAD, HALF = 2, 2048, 1024, 16, 64, 32
SCALE = HALF ** -0.5
LAMBDA_INIT = 0.8 - 0.6 * float(np.exp(-0.3 * (2 - 1)))
EPS = 1e-5
NCORES = 8
HPC = H // NCORES          # 2 heads per core
DC = HPC * HEAD            # 128 feature dims per core
NT = B * N                 # 4096 tokens
NTC = NT // 512            # 8 token chunks of 512
F32 = mybir.dt.float32

MM_DT = {"float32r": mybir.dt.float32r, "float32": mybir.dt.float32}[
    os.environ.get("BASS_MM_DT", "float32r")
]

LAST_EXEC_NS = None
_PROG_CACHE = {}


def _build_kernel(nc):
    AF = mybir.ActivationFunctionType
    ALU = mybir.AluOpType
    AX = mybir.AxisListType

    x1T = nc.dram_tensor("x1T", (DIM, NT), MM_DT, kind="ExternalInput").ap()
    x2T = nc.dram_tensor("x2T", (DIM, NT), MM_DT, kind="ExternalInput").ap()
    wqT = nc.dram_tensor("wqT", (DIM, DC), MM_DT, kind="ExternalInput").ap()
    wkT = nc.dram_tensor("wkT", (DIM, DC), MM_DT, kind="ExternalInput").ap()
    wvT = nc.dram_tensor("wvT", (DIM, DC), MM_DT, kind="ExternalInput").ap()
    woT = nc.dram_tensor("woT", (DC, DIM), MM_DT, kind="ExternalInput").ap()
    bqv = nc.dram_tensor("bqv", (DC, 1), F32, kind="ExternalInput").ap()
    bkv = nc.dram_tensor("bkv", (DC, 1), F32, kind="ExternalInput").ap()
    bvb = nc.dram_tensor("bvb", (128, DC), F32, kind="ExternalInput").ap()
    lamn = nc.dram_tensor("lamn", (128, HPC), F32, kind="ExternalInput").ap()
    vones = nc.dram_tensor("vones", (128, 32, 2), MM_DT, kind="ExternalInput").ap()
    gwv = nc.dram_tensor("gwv", (DC, 1), F32, kind="ExternalInput").ap()
    gbv = nc.dram_tensor("gbv", (DC, 1), F32, kind="ExternalInput").ap()
    out1p = nc.dram_tensor("out1p", (NT, DIM), F32, kind="ExternalOutput").ap()
    out2p = nc.dram_tensor("out2p", (NT, DIM), F32, kind="ExternalOutput").ap()

    with tile.TileContext(nc) as tc, ExitStack() as top:
        consts = top.enter_context(tc.tile_pool(name="consts", bufs=1))
        qkpool = top.enter_context(tc.tile_pool(name="qkpool", bufs=1))
        vpool = top.enter_context(tc.tile_pool(name="vpool", bufs=1))

        # ---- constants ----
        ident = consts.tile([128, 128], F32, tag="ident")
        make_identity(nc, ident)
        ones_p = consts.tile([128, 1], F32, tag="ones_p")
        nc.vector.memset(ones_p, 1.0)
        ones_r = consts.tile([1, 128], F32, tag="ones_r")
        nc.vector.memset(ones_r, 1.0)
        wq_t = consts.tile([128, 8, DC], MM_DT, tag="wq")
        wk_t = consts.tile([128, 8, DC], MM_DT, tag="wk")
        wv_t = consts.tile([128, 8, DC], MM_DT, tag="wv")
        nc.sync.dma_start(wq_t, wqT.rearrange("(kc p) d -> p kc d", p=128))
        nc.sync.dma_start(wk_t, wkT.rearrange("(kc p) d -> p kc d", p=128))
        nc.sync.dma_start(wv_t, wvT.rearrange("(kc p) d -> p kc d", p=128))
        wo_t = consts.tile([DC, DIM], MM_DT, tag="wo")
        nc.sync.dma_start(wo_t, woT)
        bq_t = consts.tile([DC, 1], F32, tag="bq")
        bk_t = consts.tile([DC, 1], F32, tag="bk")
        nc.sync.dma_start(bq_t, bqv)
        nc.sync.dma_start(bk_t, bkv)
        bvb_t = consts.tile([128, DC], F32, tag="bvb")
        nc.sync.dma_start(bvb_t, bvb)
        lam_t = consts.tile([128, HPC], F32, tag="lam")
        nc.sync.dma_start(lam_t, lamn)
        gw_t = consts.tile([DC, 1], F32, tag="gw")
        gb_t = consts.tile([DC, 1], F32, tag="gb")
        nc.sync.dma_start(gw_t, gwv)
        nc.sync.dma_start(gb_t, gbv)
        eps_t = consts.tile([1, 1], F32, tag="eps")
        nc.vector.memset(eps_t, EPS)

        # persistent activations
        q1_t = qkpool.tile([128, NT], MM_DT, tag="q1")
        k1_t = qkpool.tile([128, NT], MM_DT, tag="k1")
        q2_t = qkpool.tile([128, NT], MM_DT, tag="q2")
        k2_t = qkpool.tile([128, NT], MM_DT, tag="k2")
        # V token-major: (tok 128, chunk 32, [64 h0 | 1 | 64 h1 | 1])
        v1_t = vpool.tile([128, 32, 130], MM_DT, tag="v1")
        v2_t = vpool.tile([128, 32, 130], MM_DT, tag="v2")

        # ================= phase P: projections =================
        with ExitStack() as ph:
            xin = ph.enter_context(tc.tile_pool(name="xin", bufs=2))
            pqk = ph.enter_context(tc.tile_pool(name="pqk", bufs=3, space="PSUM"))
            pv = ph.enter_context(tc.tile_pool(name="pv", bufs=4, space="PSUM"))
            for xi, (xT, qd, kd, vd) in enumerate(
                ((x1T, q1_t, k1_t, v1_t), (x2T, q2_t, k2_t, v2_t))
            ):
                xTr = xT.rearrange("(kc p) t -> p kc t", p=128)
                for tcn in range(NTC):
                    ts0 = tcn * 512
                    xt = xin.tile([128, 8, 512], MM_DT, tag="x")
                    nc.sync.dma_start(xt, xTr[:, :, ts0 : ts0 + 512])
                    for wt, bt, dst in ((wq_t, bq_t, qd), (wk_t, bk_t, kd)):
                        ps = pqk.tile([128, 512], F32, tag="qk")
                        for kc in range(8):
                            nc.tensor.matmul(
                                ps,
                                lhsT=wt[:, kc, :],
                                rhs=xt[:, kc, :],
                                start=(kc == 0),
                                stop=(kc == 7),
                            )
                        # bias add + PSUM->SBUF in one pass
                        nc.vector.tensor_scalar(
                            dst[:, ts0 : ts0 + 512], ps, bt, None, ALU.add
                        )
                    for sc in range(4):
                        psv = pv.tile([128, 128], F32, tag="v")
                        for kc in range(8):
                            nc.tensor.matmul(
                                psv,
                                lhsT=xt[:, kc, sc * 128 : (sc + 1) * 128],
                                rhs=wv_t[:, kc, :],
                                start=(kc == 0),
                                stop=(kc == 7),
                            )
                        sg = tcn * 4 + sc
                        nc.vector.tensor_add(
                            vd[:, sg, 0:64], psv[:, 0:64], bvb_t[:, 0:64]
                        )
                        nc.vector.tensor_add(
                            vd[:, sg, 65:129], psv[:, 64:128], bvb_t[:, 64:128]
                        )
            for vd in (v1_t, v2_t):
                nc.sync.dma_start(vd[:, :, 64:65], vones[:, :, 0:1])
                nc.sync.dma_start(vd[:, :, 129:130], vones[:, :, 1:2])

        # ================= phase A: attention =================
        with ExitStack() as ph:
            pp = ph.enter_context(tc.tile_pool(name="pp", bufs=4))
            osb = ph.enter_context(tc.tile_pool(name="osb", bufs=4))
            ocomb = ph.enter_context(tc.tile_pool(name="ocomb", bufs=2))
            otp = ph.enter_context(tc.tile_pool(name="otp", bufs=2))
            wos = ph.enter_context(tc.tile_pool(name="wos", bufs=4))
            small = ph.enter_context(tc.tile_pool(name="small", bufs=8))
            ps_s = ph.enter_context(tc.tile_pool(name="ps_s", bufs=2, space="PSUM"))
            ps_o = ph.enter_context(tc.tile_pool(name="ps_o", bufs=2, space="PSUM"))
            ps_t = ph.enter_context(tc.tile_pool(name="ps_t", bufs=2, space="PSUM"))

            for b in range(B):
                for a in range(2):
                    qt = q1_t if a == 0 else q2_t
                    kt = k2_t if a == 0 else k1_t
                    vt = v2_t if a == 0 else v1_t
                    outp = out1p if a == 0 else out2p
                    ot = otp.tile([128, N], MM_DT, tag="ot")
                    # combined, normalized attn output for BOTH heads:
                    # (q 128, sub 16, [h0 64 | h1 64])
                    ocu = ocomb.tile([128, 16, 128], F32, tag="oc")
                    sdv = small.tile([128, 1], F32, tag="sdv")
                    tdv = small.tile([128, 1], F32, tag="tdv")
                    for h in range(HPC):
                        hb = h * 64
                        st = small.tile([128, 2], F32, tag="st")
                        for qc in range(4):
                            q0 = b * N + qc * 512
                            o1 = ps_o.tile([65, 512], F32, tag="o")
                            o2 = ps_o.tile([65, 512], F32, tag="o")
                            for kg in range(8):
                                s1 = ps_s.tile([128, 1024], F32, tag="s")
                                s2 = ps_s.tile([128, 1024], F32, tag="s")
                                for i2 in range(2):
                                    kc = kg * 2 + i2
                                    k0 = b * N + kc * 128
                                    for s_t, hh in ((s1, hb), (s2, hb + 32)):
                                        nc.tensor.matmul(
                                            s_t[:, i2 * 512 : (i2 + 1) * 512],
                                            lhsT=kt[hh : hh + 32, k0 : k0 + 128],
                                            rhs=qt[hh : hh + 32, q0 : q0 + 512],
                                            start=True,
                                            stop=True,
                                            tile_position=(hh, 0),
                                        )
                                p1 = pp.tile([128, 1024], MM_DT, tag="p")
                                p2 = pp.tile([128, 1024], MM_DT, tag="p")
                                nc.scalar.activation(p1, s1, AF.Exp, scale=SCALE)
                                nc.scalar.activation(p2, s2, AF.Exp, scale=SCALE)
                                va0 = h * 65  # [0:65] = h0 dims+ones, [65:130] = h1
                                for i2 in range(2):
                                    kc = kg * 2 + i2
                                    vs = b * 16 + kc
                                    first = kg == 0 and i2 == 0
                                    last = kg == 7 and i2 == 1
                                    nc.tensor.matmul(
                                        o1,
                                        lhsT=vt[:, vs, va0 : va0 + 65],
                                        rhs=p1[:, i2 * 512 : (i2 + 1) * 512],
                                        start=first,
                                        stop=last,
                                        skip_group_check=True,
                                    )
                                    nc.tensor.matmul(
                                        o2,
                                        lhsT=vt[:, vs, va0 : va0 + 65],
                                        rhs=p2[:, i2 * 512 : (i2 + 1) * 512],
                                        start=first,
                                        stop=last,
                                        skip_group_check=True,
                                    )
                            # evacuate O psum, transpose to (q, 65)
                            o1s = osb.tile([65, 512], F32, tag="ob")
                            o2s = osb.tile([65, 512], F32, tag="ob")
                            nc.vector.tensor_copy(o1s, o1)
                            nc.vector.tensor_copy(o2s, o2)
                            t1 = ps_t.tile([128, 260], F32, tag="t")
                            t2 = ps_t.tile([128, 260], F32, tag="t")
                            for sub in range(4):
                                nc.tensor.transpose(
                                    t1[:, sub * 65 : (sub + 1) * 65],
                                    o1s[:, sub * 128 : (sub + 1) * 128],
                                    ident[0:65, 0:65],
                                )
                                nc.tensor.transpose(
                                    t2[:, sub * 65 : (sub + 1) * 65],
                                    o2s[:, sub * 128 : (sub + 1) * 128],
                                    ident[0:65, 0:65],
                                )
                            for sub in range(4):
                                c0 = sub * 65
                                r1 = small.tile([128, 1], F32, tag="r1")
                                r2 = small.tile([128, 1], F32, tag="r2")
                                s2p = small.tile([128, 1], F32, tag="s2p")
                                nc.vector.reciprocal(r1, t1[:, c0 + 64 : c0 + 65])
                                nc.vector.reciprocal(r2, t2[:, c0 + 64 : c0 + 65])
                                nc.vector.tensor_tensor(
                                    s2p, r2, lam_t[:, h : h + 1], ALU.mult
                                )
                                sidx = qc * 4 + sub
                                tmp = small.tile([128, 64], F32, tag="tmp")
                                nc.vector.tensor_scalar(
                                    tmp, t1[:, c0 : c0 + 64], r1, None, ALU.mult
                                )
                                # oc = (O2 * s2p) + tmp   (s2p = -lam / sum2)
                                nc.vector.scalar_tensor_tensor(
                                    ocu[:, sidx, hb : hb + 64],
                                    t2[:, c0 : c0 + 64],
                                    s2p,
                                    tmp,
                                    ALU.mult,
                                    ALU.add,
                                )
                        # ---- GroupNorm stats over the whole unit ----
                        sq = ocomb.tile([128, 16, 64], F32, tag="sq")
                        nc.vector.reduce_sum(
                            st[:, 0:1], ocu[:, :, hb : hb + 64], axis=AX.XY
                        )
                        nc.vector.tensor_tensor(
                            sq, ocu[:, :, hb : hb + 64], ocu[:, :, hb : hb + 64],
                            ALU.mult,
                        )
                        nc.vector.reduce_sum(st[:, 1:2], sq, axis=AX.XY)
                        red = ps_t.tile([1, 2], F32, tag="t")
                        nc.tensor.matmul(red, lhsT=ones_p, rhs=st, start=True, stop=True)
                        mr = small.tile([1, 2], F32, tag="mr")
                        nc.vector.tensor_scalar(
                            mr, red, 1.0 / (N * HEAD), None, ALU.mult
                        )
                        m2 = small.tile([1, 1], F32, tag="m2")
                        nc.vector.tensor_tensor(m2, mr[:, 0:1], mr[:, 0:1], ALU.mult)
                        var = small.tile([1, 1], F32, tag="var")
                        nc.vector.tensor_sub(var, mr[:, 1:2], m2)
                        sd = small.tile([1, 1], F32, tag="sd")
                        nc.scalar.activation(sd, var, AF.Sqrt, bias=eps_t)
                        mrs = small.tile([1, 2], F32, tag="mrs")
                        nc.vector.tensor_copy(mrs[:, 0:1], mr[:, 0:1])
                        nc.vector.reciprocal(mrs[:, 1:2], sd)
                        # broadcast [mean, rstd] to all 128 partitions
                        bc = ps_t.tile([128, 2], F32, tag="t")
                        nc.tensor.matmul(bc, lhsT=ones_r, rhs=mrs, start=True, stop=True)
                        hs = slice(hb, hb + 64)
                        tmp1 = small.tile([128, 1], F32, tag="tmp1")
                        nc.vector.tensor_tensor(
                            sdv[hs, :], gw_t[hs, :], bc[hs, 1:2], ALU.mult
                        )
                        nc.vector.tensor_tensor(
                            tmp1[hs, :], bc[hs, 0:1], sdv[hs, :], ALU.mult
                        )
                        nc.vector.tensor_sub(tdv[hs, :], gb_t[hs, :], tmp1[hs, :])
                    # transpose back + affine into the (b, a) stacked tile
                    for sub in range(16):
                        tb = ps_t.tile([128, 128], F32, tag="t")
                        nc.tensor.transpose(tb, ocu[:, sub, :], ident)
                        for h in range(HPC):
                            hs = slice(h * 64, h * 64 + 64)
                            nc.vector.tensor_scalar(
                                ot[hs, sub * 128 : (sub + 1) * 128],
                                tb[hs, :],
                                sdv[hs, :],
                                tdv[hs, :],
                                ALU.mult,
                                ALU.add,
                            )
                    # ---- output projection for this (b, attend) ----
                    for tl in range(16):
                        for nc2 in range(2):
                            wp = ps_t.tile([128, 512], F32, tag="t")
                            nc.tensor.matmul(
                                wp,
                                lhsT=ot[:, tl * 128 : (tl + 1) * 128],
                                rhs=wo_t[:, nc2 * 512 : (nc2 + 1) * 512],
                                start=True,
                                stop=True,
                            )
                            ws = wos.tile([128, 512], F32, tag="ws")
                            nc.vector.tensor_copy(ws, wp)
                            nc.sync.dma_start(
                                outp[
                                    b * N + tl * 128 : b * N + (tl + 1) * 128,
                                    nc2 * 512 : (nc2 + 1) * 512,
                                ],
                                ws,
                            )
    return nc


def _get_program():
    key = ("prog", str(MM_DT))
    if key not in _PROG_CACHE:
        nc = bacc.Bacc("TRN2", target_bir_lowering=False, debug=False)
        _build_kernel(nc)
        nc.compile()
        _PROG_CACHE[key] = nc
    return _PROG_CACHE[key]


def _host_prep(x1, x2, Wq, bq, Wk, bk, Wv, bv, Wo, bo,
               lq1, lk1, lq2, lk2, gn_w, gn_b):
    f32 = np.float32
    x1 = np.asarray(x1, f32)
    x2 = np.asarray(x2, f32)
    lam = (
        np.exp((np.asarray(lq1, f32) * np.asarray(lk1, f32)).sum(-1))
        - np.exp((np.asarray(lq2, f32) * np.asarray(lk2, f32)).sum(-1))
        + f32(LAMBDA_INIT)
    ).astype(f32)  # (H,)
    sc = f32(1.0 - LAMBDA_INIT)
    gw = (np.asarray(gn_w, f32) * sc).reshape(H, HEAD)
    gb = (np.asarray(gn_b, f32) * sc).reshape(H, HEAD)
    Wq, Wk, Wv, Wo = (np.asarray(w, f32) for w in (Wq, Wk, Wv, Wo))
    bq, bk, bv, bo = (np.asarray(v_, f32) for v_ in (bq, bk, bv, bo))

    x1T = np.ascontiguousarray(x1.reshape(NT, DIM).T)
    x2T = np.ascontiguousarray(x2.reshape(NT, DIM).T)

    in_maps = []
    for c in range(NCORES):
        dlo, dhi = c * DC, (c + 1) * DC
        h0 = c * HPC
        in_maps.append(
            {
                "x1T": x1T,
                "x2T": x2T,
                "wqT": np.ascontiguousarray(Wq[dlo:dhi, :].T),
                "wkT": np.ascontiguousarray(Wk[dlo:dhi, :].T),
                "wvT": np.ascontiguousarray(Wv[dlo:dhi, :].T),
                "woT": np.ascontiguousarray(Wo[:, dlo:dhi].T),
                "bqv": np.ascontiguousarray(bq[dlo:dhi].reshape(DC, 1)),
                "bkv": np.ascontiguousarray(bk[dlo:dhi].reshape(DC, 1)),
                "bvb": np.ascontiguousarray(
                    np.broadcast_to(bv[dlo:dhi][None, :], (128, DC))
                ),
                "lamn": np.ascontiguousarray(
                    np.broadcast_to((-lam[h0 : h0 + HPC])[None, :], (128, HPC))
                ),
                "vones": np.ones((128, 32, 2), f32),
                "gwv": np.ascontiguousarray(
                    gw[h0 : h0 + HPC].reshape(DC, 1)
                ),
                "gbv": np.ascontiguousarray(
                    gb[h0 : h0 + HPC].reshape(DC, 1)
                ),
            }
        )

    def finish(results):
        o1 = np.zeros((NT, DIM), np.float64)
        o2 = np.zeros((NT, DIM), np.float64)
        for r in results:
            o1 += r["out1p"]
            o2 += r["out2p"]
        o1 = (o1 + bo).astype(f32).reshape(B, N, DIM)
        o2 = (o2 + bo).astype(f32).reshape(B, N, DIM)
        return o1, o2

    return in_maps, finish


def kernel(x1, x2, Wq, bq, Wk, bk, Wv, bv, Wo, bo,
           lq1, lk1, lq2, lk2, gn_w, gn_b):
    global LAST_EXEC_NS
    in_maps, finish = _host_prep(
        x1, x2, Wq, bq, Wk, bk, Wv, bv, Wo, bo,
        lq1, lk1, lq2, lk2, gn_w, gn_b,
    )
    nc = _get_program()
    trace = os.environ.get("BASS_KERNEL_TRACE", "0") == "1"
    res = run_bass_kernel_spmd(
        nc, in_maps, core_ids=list(range(NCORES)), trace=trace
    )
    LAST_EXEC_NS = res.exec_time_ns
    return finish(res.results)


# revision 25
# speedup vs baseline: 1.0438x; 1.0438x over previous
"""Differential cross-attention Bass kernel for 8 Trainium2 NeuronCores.

Sharding: heads are split across cores (2 of 16 heads per core). Each core
computes Q/K/V projections for its head slice only, both N x N differential
score maps for its (batch, head) units, softmax (no max-subtraction; scores
are O(1) so exp is safe), attn = a1 - lam*a2, GroupNorm per (b, h), and a
partial output projection against its 128-column slice of Wo. The host sums
the 8 partial outputs and adds the output bias.

Layouts on device (per core, P = SBUF partition dim):
  Q, K   dim-major   (128 dims = 2 heads x 64, B*N tokens)
  V      token-major (128 tokens/chunk, 32 chunks, 2x(64 dims + ones col))
  scores S.T         (128 k-tokens, q free)  -> exp -> P.T
  O_aug.T = V_aug.T @ P.T  (65, q)  row 64 = softmax denominator
  transpose -> (q, 65), normalize+combine per-partition, GN stats,
  transpose back -> (64, q) stacked per (b, attend) -> Wo matmul -> partial out
"""

import os
import sys
from contextlib import ExitStack

import numpy as np

for _p in ("/opt/trn_rl_repo", "/opt/pypackages"):
    if os.path.isdir(_p) and _p not in sys.path:
        sys.path.append(_p)

import concourse.bass as bass
import concourse.tile as tile
from concourse import bacc, mybir
from concourse.bass_utils import run_bass_kernel_spmd
from concourse.masks import make_identity

# ---- problem constants (hardcoded per contest contract) ----
B, N, DIM, H, HEAD, HALF = 2, 2048, 1024, 16, 64, 32
SCALE = HALF ** -0.5
LAMBDA_INIT = 0.8 - 0.6 * float(np.exp(-0.3 * (2 - 1)))
EPS = 1e-5
NCORES = 8
HPC = H // NCORES          # 2 heads per core
DC = HPC * HEAD            # 128 feature dims per core
NT = B * N                 # 4096 tokens
NTC = NT // 512            # 8 token chunks of 512
F32 = mybir.dt.float32

MM_DT = {"float32r": mybir.dt.float32r, "float32": mybir.dt.float32}[
    os.environ.get("BASS_MM_DT", "float32r")
]

LAST_EXEC_NS = None
_PROG_CACHE = {}


def _build_kernel(nc):
    AF = mybir.ActivationFunctionType
    ALU = mybir.AluOpType
    AX = mybir.AxisListType

    x1T = nc.dram_tensor("x1T", (DIM, NT), MM_DT, kind="ExternalInput").ap()
    x2T = nc.dram_tensor("x2T", (DIM, NT), MM_DT, kind="ExternalInput").ap()
    wqT = nc.dram_tensor("wqT", (DIM, DC), MM_DT, kind="ExternalInput").ap()
    wkT = nc.dram_tensor("wkT", (DIM, DC), MM_DT, kind="ExternalInput").ap()
    wvT = nc.dram_tensor("wvT", (DIM, DC), MM_DT, kind="ExternalInput").ap()
    woT = nc.dram_tensor("woT", (DC, DIM), MM_DT, kind="ExternalInput").ap()
    bqv = nc.dram_tensor("bqv", (DC, 1), F32, kind="ExternalInput").ap()
    bkv = nc.dram_tensor("bkv", (DC, 1), F32, kind="ExternalInput").ap()
    bvb = nc.dram_tensor("bvb", (128, DC), F32, kind="ExternalInput").ap()
    lamn = nc.dram_tensor("lamn", (128, HPC), F32, kind="ExternalInput").ap()
    vones = nc.dram_tensor("vones", (128, 32, 2), MM_DT, kind="ExternalInput").ap()
    gwv = nc.dram_tensor("gwv", (DC, 1), F32, kind="ExternalInput").ap()
    gbv = nc.dram_tensor("gbv", (DC, 1), F32, kind="ExternalInput").ap()
    out1p = nc.dram_tensor("out1p", (NT, DIM), F32, kind="ExternalOutput").ap()
    out2p = nc.dram_tensor("out2p", (NT, DIM), F32, kind="ExternalOutput").ap()

    with tile.TileContext(nc) as tc, ExitStack() as top:
        consts = top.enter_context(tc.tile_pool(name="consts", bufs=1))
        qkpool = top.enter_context(tc.tile_pool(name="qkpool", bufs=1))
        vpool = top.enter_context(tc.tile_pool(name="vpool", bufs=1))

        # ---- constants ----
        ident = consts.tile([128, 128], F32, tag="ident")
        make_identity(nc, ident)
        ones_p = consts.tile([128, 1], F32, tag="ones_p")
        nc.vector.memset(ones_p, 1.0)
        ones_r = consts.tile([1, 128], F32, tag="ones_r")
        nc.vector.memset(ones_r, 1.0)
        wq_t = consts.tile([128, 8, DC], MM_DT, tag="wq")
        wk_t = consts.tile([128, 8, DC], MM_DT, tag="wk")
        wv_t = consts.tile([128, 8, DC], MM_DT, tag="wv")
        nc.sync.dma_start(wq_t, wqT.rearrange("(kc p) d -> p kc d", p=128))
        nc.sync.dma_start(wk_t, wkT.rearrange("(kc p) d -> p kc d", p=128))
        nc.sync.dma_start(wv_t, wvT.rearrange("(kc p) d -> p kc d", p=128))
        wo_t = consts.tile([DC, DIM], MM_DT, tag="wo")
        nc.sync.dma_start(wo_t, woT)
        bq_t = consts.tile([DC, 1], F32, tag="bq")
        bk_t = consts.tile([DC, 1], F32, tag="bk")
        nc.sync.dma_start(bq_t, bqv)
        nc.sync.dma_start(bk_t, bkv)
        bvb_t = consts.tile([128, DC], F32, tag="bvb")
        nc.sync.dma_start(bvb_t, bvb)
        lam_t = consts.tile([128, HPC], F32, tag="lam")
        nc.sync.dma_start(lam_t, lamn)
        gw_t = consts.tile([DC, 1], F32, tag="gw")
        gb_t = consts.tile([DC, 1], F32, tag="gb")
        nc.sync.dma_start(gw_t, gwv)
        nc.sync.dma_start(gb_t, gbv)
        eps_t = consts.tile([1, 1], F32, tag="eps")
        nc.vector.memset(eps_t, EPS)

        # persistent activations
        q1_t = qkpool.tile([128, NT], MM_DT, tag="q1")
        k1_t = qkpool.tile([128, NT], MM_DT, tag="k1")
        q2_t = qkpool.tile([128, NT], MM_DT, tag="q2")
        k2_t = qkpool.tile([128, NT], MM_DT, tag="k2")
        # V token-major: (tok 128, chunk 32, [64 h0 | 1 | 64 h1 | 1])
        v1_t = vpool.tile([128, 32, 130], MM_DT, tag="v1")
        v2_t = vpool.tile([128, 32, 130], MM_DT, tag="v2")

        # ================= phase P: projections =================
        with ExitStack() as ph:
            xin = ph.enter_context(tc.tile_pool(name="xin", bufs=2))
            pqk = ph.enter_context(tc.tile_pool(name="pqk", bufs=3, space="PSUM"))
            pv = ph.enter_context(tc.tile_pool(name="pv", bufs=4, space="PSUM"))
            for xi, (xT, qd, kd, vd) in enumerate(
                ((x1T, q1_t, k1_t, v1_t), (x2T, q2_t, k2_t, v2_t))
            ):
                xTr = xT.rearrange("(kc p) t -> p kc t", p=128)
                for tcn in range(NTC):
                    ts0 = tcn * 512
                    xt = xin.tile([128, 8, 512], MM_DT, tag="x")
                    nc.sync.dma_start(xt, xTr[:, :, ts0 : ts0 + 512])
                    for wt, bt, dst in ((wq_t, bq_t, qd), (wk_t, bk_t, kd)):
                        ps = pqk.tile([128, 512], F32, tag="qk")
                        for kc in range(8):
                            nc.tensor.matmul(
                                ps,
                                lhsT=wt[:, kc, :],
                                rhs=xt[:, kc, :],
                                start=(kc == 0),
                                stop=(kc == 7),
                            )
                        # bias add + PSUM->SBUF in one pass
                        nc.vector.tensor_scalar(
                            dst[:, ts0 : ts0 + 512], ps, bt, None, ALU.add
                        )
                    for sc in range(4):
                        psv = pv.tile([128, 128], F32, tag="v")
                        for kc in range(8):
                            nc.tensor.matmul(
                                psv,
                                lhsT=xt[:, kc, sc * 128 : (sc + 1) * 128],
                                rhs=wv_t[:, kc, :],
                                start=(kc == 0),
                                stop=(kc == 7),
                            )
                        sg = tcn * 4 + sc
                        nc.vector.tensor_add(
                            vd[:, sg, 0:64], psv[:, 0:64], bvb_t[:, 0:64]
                        )
                        nc.vector.tensor_add(
                            vd[:, sg, 65:129], psv[:, 64:128], bvb_t[:, 64:128]
                        )
            for vd in (v1_t, v2_t):
                nc.sync.dma_start(vd[:, :, 64:65], vones[:, :, 0:1])
                nc.sync.dma_start(vd[:, :, 129:130], vones[:, :, 1:2])

        # ================= phase A: attention =================
        with ExitStack() as ph:
            pp = ph.enter_context(tc.tile_pool(name="pp", bufs=4))
            osb = ph.enter_context(tc.tile_pool(name="osb", bufs=4))
            ocomb = ph.enter_context(tc.tile_pool(name="ocomb", bufs=2))
            otp = ph.enter_context(tc.tile_pool(name="otp", bufs=2))
            wos = ph.enter_context(tc.tile_pool(name="wos", bufs=4))
            small = ph.enter_context(tc.tile_pool(name="small", bufs=8))
            ps_s = ph.enter_context(tc.tile_pool(name="ps_s", bufs=3, space="PSUM"))
            ps_o = ph.enter_context(tc.tile_pool(name="ps_o", bufs=2, space="PSUM"))
            ps_t = ps_s  # share the 3 "s" slots (PSUM is only 8 banks)

            for b in range(B):
                for a in range(2):
                    qt = q1_t if a == 0 else q2_t
                    kt = k2_t if a == 0 else k1_t
                    vt = v2_t if a == 0 else v1_t
                    outp = out1p if a == 0 else out2p
                    ot = otp.tile([128, N], MM_DT, tag="ot")
                    # combined, normalized attn output for BOTH heads:
                    # (q 128, sub 16, [h0 64 | h1 64])
                    ocu = ocomb.tile([128, 16, 128], F32, tag="oc")
                    sdv = small.tile([128, 1], F32, tag="sdv")
                    tdv = small.tile([128, 1], F32, tag="tdv")
                    for h in range(HPC):
                        hb = h * 64
                        st = small.tile([128, 2], F32, tag="st")
                        for qc in range(4):
                            q0 = b * N + qc * 512
                            o1 = ps_o.tile([65, 512], F32, tag="o")
                            o2 = ps_o.tile([65, 512], F32, tag="o")
                            va0 = h * 65  # [0:65] = h0 dims+ones, [65:130] = h1

                            # software-pipelined: S(kc) scores -> exp(kc) ->
                            # @V(kc-1), so PE never stalls on ACT.
                            ptiles = [None] * 16

                            def emit_scores(kc):
                                k0 = b * N + kc * 128
                                s = ps_s.tile([128, 1024], F32, tag="s")
                                for j, hh in enumerate((hb, hb + 32)):
                                    nc.tensor.matmul(
                                        s[:, j * 512 : (j + 1) * 512],
                                        lhsT=kt[hh : hh + 32, k0 : k0 + 128],
                                        rhs=qt[hh : hh + 32, q0 : q0 + 512],
                                        start=True,
                                        stop=True,
                                        tile_position=(hh, 0),
                                    )
                                p = pp.tile([128, 1024], MM_DT, tag="p")
                                nc.scalar.activation(p, s, AF.Exp, scale=SCALE)
                                ptiles[kc] = p

                            def emit_av(kc):
                                vs = b * 16 + kc
                                p = ptiles[kc]
                                for o_t, j in ((o1, 0), (o2, 1)):
                                    nc.tensor.matmul(
                                        o_t,
                                        lhsT=vt[:, vs, va0 : va0 + 65],
                                        rhs=p[:, j * 512 : (j + 1) * 512],
                                        start=(kc == 0),
                                        stop=(kc == 15),
                                        skip_group_check=True,
                                    )

                            emit_scores(0)
                            for kc in range(1, 16):
                                emit_scores(kc)
                                emit_av(kc - 1)
                            emit_av(15)
                            # evacuate O psum, transpose to (q, 65)
                            o1s = osb.tile([65, 512], F32, tag="ob")
                            o2s = osb.tile([65, 512], F32, tag="ob")
                            nc.vector.tensor_copy(o1s, o1)
                            nc.vector.tensor_copy(o2s, o2)
                            t1 = ps_t.tile([128, 260], F32, tag="s")
                            t2 = ps_t.tile([128, 260], F32, tag="s")
                            for sub in range(4):
                                nc.tensor.transpose(
                                    t1[:, sub * 65 : (sub + 1) * 65],
                                    o1s[:, sub * 128 : (sub + 1) * 128],
                                    ident[0:65, 0:65],
                                )
                                nc.tensor.transpose(
                                    t2[:, sub * 65 : (sub + 1) * 65],
                                    o2s[:, sub * 128 : (sub + 1) * 128],
                                    ident[0:65, 0:65],
                                )
                            for sub in range(4):
                                c0 = sub * 65
                                r1 = small.tile([128, 1], F32, tag="r1")
                                r2 = small.tile([128, 1], F32, tag="r2")
                                s2p = small.tile([128, 1], F32, tag="s2p")
                                nc.vector.reciprocal(r1, t1[:, c0 + 64 : c0 + 65])
                                nc.vector.reciprocal(r2, t2[:, c0 + 64 : c0 + 65])
                                nc.vector.tensor_tensor(
                                    s2p, r2, lam_t[:, h : h + 1], ALU.mult
                                )
                                sidx = qc * 4 + sub
                                tmp = small.tile([128, 64], F32, tag="tmp")
                                nc.vector.tensor_scalar(
                                    tmp, t1[:, c0 : c0 + 64], r1, None, ALU.mult
                                )
                                # oc = (O2 * s2p) + tmp   (s2p = -lam / sum2)
                                nc.vector.scalar_tensor_tensor(
                                    ocu[:, sidx, hb : hb + 64],
                                    t2[:, c0 : c0 + 64],
                                    s2p,
                                    tmp,
                                    ALU.mult,
                                    ALU.add,
                                )
                        # ---- GroupNorm stats over the whole unit ----
                        sq = ocomb.tile([128, 16, 64], F32, tag="sq")
                        nc.vector.reduce_sum(
                            st[:, 0:1], ocu[:, :, hb : hb + 64], axis=AX.XY
                        )
                        nc.vector.tensor_tensor(
                            sq, ocu[:, :, hb : hb + 64], ocu[:, :, hb : hb + 64],
                            ALU.mult,
                        )
                        nc.vector.reduce_sum(st[:, 1:2], sq, axis=AX.XY)
                        red = ps_t.tile([1, 2], F32, tag="s")
                        nc.tensor.matmul(red, lhsT=ones_p, rhs=st, start=True, stop=True)
                        mr = small.tile([1, 2], F32, tag="mr")
                        nc.vector.tensor_scalar(
                            mr, red, 1.0 / (N * HEAD), None, ALU.mult
                        )
                        m2 = small.tile([1, 1], F32, tag="m2")
                        nc.vector.tensor_tensor(m2, mr[:, 0:1], mr[:, 0:1], ALU.mult)
                        var = small.tile([1, 1], F32, tag="var")
                        nc.vector.tensor_sub(var, mr[:, 1:2], m2)
                        sd = small.tile([1, 1], F32, tag="sd")
                        nc.scalar.activation(sd, var, AF.Sqrt, bias=eps_t)
                        mrs = small.tile([1, 2], F32, tag="mrs")
                        nc.vector.tensor_copy(mrs[:, 0:1], mr[:, 0:1])
                        nc.vector.reciprocal(mrs[:, 1:2], sd)
                        # broadcast [mean, rstd] to all 128 partitions
                        bc = ps_t.tile([128, 2], F32, tag="s")
                        nc.tensor.matmul(bc, lhsT=ones_r, rhs=mrs, start=True, stop=True)
                        hs = slice(hb, hb + 64)
                        tmp1 = small.tile([128, 1], F32, tag="tmp1")
                        nc.vector.tensor_tensor(
                            sdv[hs, :], gw_t[hs, :], bc[hs, 1:2], ALU.mult
                        )
                        nc.vector.tensor_tensor(
                            tmp1[hs, :], bc[hs, 0:1], sdv[hs, :], ALU.mult
                        )
                        nc.vector.tensor_sub(tdv[hs, :], gb_t[hs, :], tmp1[hs, :])
                    # transpose back + affine into the (b, a) stacked tile
                    for sub in range(16):
                        tb = ps_t.tile([128, 128], F32, tag="s")
                        nc.tensor.transpose(tb, ocu[:, sub, :], ident)
                        for h in range(HPC):
                            hs = slice(h * 64, h * 64 + 64)
                            nc.vector.tensor_scalar(
                                ot[hs, sub * 128 : (sub + 1) * 128],
                                tb[hs, :],
                                sdv[hs, :],
                                tdv[hs, :],
                                ALU.mult,
                                ALU.add,
                            )
                    # ---- output projection for this (b, attend) ----
                    for tl in range(16):
                        for nc2 in range(2):
                            wp = ps_t.tile([128, 512], F32, tag="s")
                            nc.tensor.matmul(
                                wp,
                                lhsT=ot[:, tl * 128 : (tl + 1) * 128],
                                rhs=wo_t[:, nc2 * 512 : (nc2 + 1) * 512],
                                start=True,
                                stop=True,
                            )
                            ws = wos.tile([128, 512], F32, tag="ws")
                            nc.vector.tensor_copy(ws, wp)
                            nc.sync.dma_start(
                                outp[
                                    b * N + tl * 128 : b * N + (tl + 1) * 128,
                                    nc2 * 512 : (nc2 + 1) * 512,
                                ],
                                ws,
                            )
    return nc


def _get_program():
    key = ("prog", str(MM_DT))
    if key not in _PROG_CACHE:
        nc = bacc.Bacc("TRN2", target_bir_lowering=False, debug=False)
        _build_kernel(nc)
        nc.compile()
        _PROG_CACHE[key] = nc
    return _PROG_CACHE[key]


def _host_prep(x1, x2, Wq, bq, Wk, bk, Wv, bv, Wo, bo,
               lq1, lk1, lq2, lk2, gn_w, gn_b):
    f32 = np.float32
    x1 = np.asarray(x1, f32)
    x2 = np.asarray(x2, f32)
    lam = (
        np.exp((np.asarray(lq1, f32) * np.asarray(lk1, f32)).sum(-1))
        - np.exp((np.asarray(lq2, f32) * np.asarray(lk2, f32)).sum(-1))
        + f32(LAMBDA_INIT)
    ).astype(f32)  # (H,)
    sc = f32(1.0 - LAMBDA_INIT)
    gw = (np.asarray(gn_w, f32) * sc).reshape(H, HEAD)
    gb = (np.asarray(gn_b, f32) * sc).reshape(H, HEAD)
    Wq, Wk, Wv, Wo = (np.asarray(w, f32) for w in (Wq, Wk, Wv, Wo))
    bq, bk, bv, bo = (np.asarray(v_, f32) for v_ in (bq, bk, bv, bo))

    x1T = np.ascontiguousarray(x1.reshape(NT, DIM).T)
    x2T = np.ascontiguousarray(x2.reshape(NT, DIM).T)

    in_maps = []
    for c in range(NCORES):
        dlo, dhi = c * DC, (c + 1) * DC
        h0 = c * HPC
        in_maps.append(
            {
                "x1T": x1T,
                "x2T": x2T,
                "wqT": np.ascontiguousarray(Wq[dlo:dhi, :].T),
                "wkT": np.ascontiguousarray(Wk[dlo:dhi, :].T),
                "wvT": np.ascontiguousarray(Wv[dlo:dhi, :].T),
                "woT": np.ascontiguousarray(Wo[:, dlo:dhi].T),
                "bqv": np.ascontiguousarray(bq[dlo:dhi].reshape(DC, 1)),
                "bkv": np.ascontiguousarray(bk[dlo:dhi].reshape(DC, 1)),
                "bvb": np.ascontiguousarray(
                    np.broadcast_to(bv[dlo:dhi][None, :], (128, DC))
                ),
                "lamn": np.ascontiguousarray(
                    np.broadcast_to((-lam[h0 : h0 + HPC])[None, :], (128, HPC))
                ),
                "vones": np.ones((128, 32, 2), f32),
                "gwv": np.ascontiguousarray(
                    gw[h0 : h0 + HPC].reshape(DC, 1)
                ),
                "gbv": np.ascontiguousarray(
                    gb[h0 : h0 + HPC].reshape(DC, 1)
                ),
            }
        )

    def finish(results):
        o1 = np.zeros((NT, DIM), np.float64)
        o2 = np.zeros((NT, DIM), np.float64)
        for r in results:
            o1 += r["out1p"]
            o2 += r["out2p"]
        o1 = (o1 + bo).astype(f32).reshape(B, N, DIM)
        o2 = (o2 + bo).astype(f32).reshape(B, N, DIM)
        return o1, o2

    return in_maps, finish


def kernel(x1, x2, Wq, bq, Wk, bk, Wv, bv, Wo, bo,
           lq1, lk1, lq2, lk2, gn_w, gn_b):
    global LAST_EXEC_NS
    in_maps, finish = _host_prep(
        x1, x2, Wq, bq, Wk, bk, Wv, bv, Wo, bo,
        lq1, lk1, lq2, lk2, gn_w, gn_b,
    )
    nc = _get_program()
    trace = os.environ.get("BASS_KERNEL_TRACE", "0") == "1"
    res = run_bass_kernel_spmd(
        nc, in_maps, core_ids=list(range(NCORES)), trace=trace
    )
    LAST_EXEC_NS = res.exec_time_ns
    return finish(res.results)
